# revision 13
# baseline (speedup 1.0000x reference)
"""Binarized 4-layer MLP (8192x784 -> 6144 -> 6144 -> 6144 -> 10, log_softmax)
on 8 Trainium2 NeuronCores, data-parallel over the batch.

Per-core dataflow (batch slice of 1024, feature-major activations [feat, batch]):
  fc1: x @ sign(w1).T as a 2-term fp16 hi/lo split of x, with the two terms
       stacked along the contraction dim (1568 rows -> 13 k-tiles). fp16
       upconverts losslessly to the PE's e10m11 internal format and the
       weights are exactly +-1, so this reproduces fp32 accuracy.
  fc2: one level of Strassen over the 1024x6144 @ 6144x6144 binary matmul:
       7 half-size products instead of 8. Activation-side combos are
       {-2,0,2} (exact in fp8, built on DVE); weight-side combos are
       precomputed on the host. Products run in fp8e4 DoubleRow; all
       partial sums are small integers so fp32 PSUM stays bit-exact.
  fc3: classic sign(h) @ sign(w).T in fp8e4 DoubleRowSwInterleave.
  fc4: fused into the fc3 m-loop, single fp16 pass (w4 and h3 in fp16).
  log_softmax: max-free (binarized logits are bounded, exp cannot overflow):
       out = l - ln(sum(exp(l))), with the per-column ln-sum carried in
       partition 32 of the logits tile through one PE transpose.
"""

import numpy as np
import ml_dtypes

import concourse.bass as bass
import concourse.mybir as mybir
from concourse import bacc
from concourse.tile import TileContext
from concourse.bass_utils import run_bass_kernel_spmd
from concourse.masks import make_identity

dt = mybir.dt

CORES = 8
B = 8192
BC = B // CORES          # 1024 batch rows per core
DIN = 784
KT1 = 13                 # fc1 contraction tiles: 2*784 = 1568 padded to 1664
K1P = KT1 * 128
DH = 6144
MT = DH // 128           # 48 feature tiles
KB = DH // 256           # 24 DoubleRow contraction blocks (full 6144)
KBS = 12                 # DoubleRow blocks per 3072 Strassen half
NF = 24                  # 128-wide output chunks per 3072 block-column
DOUT = 10
NH = BC // 512           # 2 moving halves of 512
MQ = 12                  # fc1 m-groups (w1 streamed per 4 m-tiles)
MPQ = MT // MQ

BF16 = ml_dtypes.bfloat16
FP8 = mybir.dt.np(dt.float8e4)

last_exec_time_ns = None


def _build_program():
    nc = bacc.Bacc("TRN2", target_bir_lowering=False, debug=False,
                   num_devices=CORES)

    xt = nc.dram_tensor("xt", [128, KT1, BC], dt.float16,
                        kind="ExternalInput").ap()
    w1t = nc.dram_tensor("w1t", [MQ, 128, KT1, MPQ * 128], dt.float16,
                         kind="ExternalInput").ap()
    w2s = nc.dram_tensor("w2s", [NF, 128, 7, KBS, 2, 128], dt.float8e4,
                         kind="ExternalInput").ap()
    w3p = nc.dram_tensor("w3p", [MT, 128, KB, 128, 2], dt.float8e4,
                         kind="ExternalInput").ap()
    w4p = nc.dram_tensor("w4p", [128, MT, DOUT], dt.float16,
                         kind="ExternalInput").ap()
    b1p = nc.dram_tensor("b1p", [128, MT], dt.float32, kind="ExternalInput").ap()
    b2p = nc.dram_tensor("b2p", [128, MT], dt.float32, kind="ExternalInput").ap()
    b3p = nc.dram_tensor("b3p", [128, MT], dt.float32, kind="ExternalInput").ap()
    b4p = nc.dram_tensor("b4p", [DOUT, 1], dt.float32, kind="ExternalInput").ap()
    out = nc.dram_tensor("out", [BC, DOUT], dt.float32, kind="ExternalOutput").ap()

    DR = mybir.MatmulPerfMode.DoubleRow
    DRS = mybir.MatmulPerfMode.DoubleRowSwInterleave
    AF = mybir.ActivationFunctionType
    ADD = mybir.AluOpType.add
    SUB = mybir.AluOpType.subtract
    MULT = mybir.AluOpType.mult

    with TileContext(nc) as tc:
        with tc.tile_pool(name="consts", bufs=1) as cpool, \
             tc.tile_pool(name="h2p", bufs=1) as h2pool:
            with tc.tile_pool(name="h1p", bufs=1) as h1pool:
                with tc.tile_pool(name="xtq", bufs=1) as xtq, \
                     tc.tile_pool(name="w1pool", bufs=3) as w1pool, \
                     tc.tile_pool(name="ps1", bufs=3, space="PSUM") as ps1:
                    # --- startup DMAs in fc1 consumption order
                    # (k-interleaved), alternating dispatch engines ---
                    w1q0 = {}
                    xt_half = {}
                    for k in range(KT1):
                        if k == 0:
                            for n in range(NH):
                                tx = xtq.tile([128, 512], dt.float16,
                                              tag=f"xt0_{n}")
                                nc.sync.dma_start(
                                    out=tx[:],
                                    in_=xt[:, 0, n * 512:(n + 1) * 512])
                                xt_half[(0, n)] = tx[:, :]
                        else:
                            tx = xtq.tile([128, BC], dt.float16, tag=f"xt_{k}")
                            nc.sync.dma_start(out=tx[:], in_=xt[:, k, :])
                            for n in range(NH):
                                xt_half[(k, n)] = tx[:, n * 512:(n + 1) * 512]
                        tw = xtq.tile([128, MPQ * 128], dt.float16,
                                      tag=f"w1q0_{k}")
                        nc.gpsimd.dma_start(out=tw[:], in_=w1t[0, :, k, :])
                        w1q0[k] = tw
                    # first weight tiles of fc2/fc3 (avoid waiting on the
                    # SBUF zone recycle at the phase boundary)
                    w2pre = {}
                    for i, eng in ((4, nc.sync), (7, nc.gpsimd)):
                        t = cpool.tile([128, KBS, 2, 128], dt.float8e4,
                                       tag=f"w2pre{i}")
                        eng.dma_start(out=t[:], in_=w2s[0, :, i - 1])
                        w2pre[i] = t
                    w3f = cpool.tile([128, KB, 128, 2], dt.float8e4)
                    nc.gpsimd.dma_start(out=w3f[:], in_=w3p[0])

                    b1_sb = cpool.tile([128, MT], dt.float32)
                    nc.sync.dma_start(out=b1_sb[:], in_=b1p[:])
                    b2_sb = cpool.tile([128, MT], dt.float32)
                    nc.gpsimd.dma_start(out=b2_sb[:], in_=b2p[:])
                    b3_sb = cpool.tile([128, MT], dt.float32)
                    nc.sync.dma_start(out=b3_sb[:], in_=b3p[:])
                    b4_sb = cpool.tile([DOUT, 1], dt.float32)
                    nc.gpsimd.dma_start(out=b4_sb[:], in_=b4p[:])
                    w4_sb = cpool.tile([128, MT, DOUT], dt.float16)
                    nc.sync.dma_start(out=w4_sb[:], in_=w4p[:])
                    ident = cpool.tile([33, 33], dt.float32)
                    make_identity(nc, ident[:])
                    ones_sb = cpool.tile([DOUT, 1], dt.float16)
                    nc.vector.memset(ones_sb[:], 1.0)
                    # pre-warm Exp/Ln activation tables
                    warm = cpool.tile([1, 1], dt.float32)
                    nc.scalar.activation(warm[:], ident[0:1, 0:1], AF.Exp)
                    nc.scalar.activation(warm[:], warm[:], AF.Ln)

                    h1 = h1pool.tile([128, MT, BC], dt.float8e4)
                    h2 = h2pool.tile([128, MT, BC], dt.float8e4)

                    # ---------------- fc1 ----------------
                    for q in range(MQ):
                        if q == 0:
                            def lhs1(k, mi):
                                return w1q0[k][:, mi * 128:(mi + 1) * 128]
                        else:
                            w1q = w1pool.tile([128, KT1, MPQ * 128],
                                              dt.float16, tag="w1")
                            nc.sync.dma_start(out=w1q[:], in_=w1t[q])

                            def lhs1(k, mi, w1q=w1q):
                                return w1q[:, k, mi * 128:(mi + 1) * 128]
                        for mi in range(MPQ):
                            m = q * MPQ + mi
                            psum = ps1.tile([128, BC], dt.float32, tag="ps1")
                            for k in range(KT1):
                                for n in range(NH):
                                    nc.tensor.matmul(
                                        psum[:, n * 512:(n + 1) * 512],
                                        lhs1(k, mi),
                                        xt_half[(k, n)],
                                        start=(k == 0),
                                        stop=(k == KT1 - 1),
                                    )
                            nc.scalar.sign(h1[:, m, :], psum[:, :],
                                           bias=b1_sb[:, m:m + 1])

                # ------------- fc2 (one-level Strassen) -------------
                with tc.tile_pool(name="s2pool", bufs=1) as spool, \
                     tc.tile_pool(name="w2pool", bufs=8) as w2pool, \
                     tc.tile_pool(name="c2pool", bufs=2) as cp2, \
                     tc.tile_pool(name="ps2", bufs=1, space="PSUM") as ps2:
                    dmae = [nc.sync, nc.gpsimd]
                    nd = [0]

                    def tt(out_, a, b_, op):
                        nc.vector.scalar_tensor_tensor(out_, a, 1.0, b_,
                                                       MULT, op)

                    ORDER = [4, 7, 5, 1, 2, 3, 6]
                    for px in range(2):
                        c0 = px * 256
                        A11 = h1[:, 0:24, c0:c0 + 256]
                        A12 = h1[:, 24:48, c0:c0 + 256]
                        A21 = h1[:, 0:24, 512 + c0:512 + c0 + 256]
                        A22 = h1[:, 24:48, 512 + c0:512 + c0 + 256]

                        def sbuild(tag, a, b_, op):
                            t = spool.tile([128, 24, 256], dt.float8e4,
                                           tag=tag)
                            tt(t[:], a, b_, op)
                            return t
                        S = {}
                        S[7] = sbuild("s7", A12, A22, SUB)
                        S[5] = sbuild("s5", A11, A12, ADD)
                        S[1] = sbuild("s1", A11, A22, ADD)
                        S[2] = sbuild("s2", A21, A22, ADD)
                        S[6] = sbuild("s6", A21, A11, SUB)

                        def moving(i, blk):
                            if i == 3:
                                return h1[:, 2 * blk:2 * blk + 2,
                                          c0:c0 + 256]
                            if i == 4:
                                return h1[:, 24 + 2 * blk:24 + 2 * blk + 2,
                                          512 + c0:512 + c0 + 256]
                            return S[i][:, 2 * blk:2 * blk + 2, :]

                        for f in range(NF):
                            wts = {}
                            for i in ORDER:
                                if px == 0 and f == 0 and i in w2pre:
                                    wts[i] = w2pre[i]
                                else:
                                    t = w2pool.tile([128, KBS, 2, 128],
                                                    dt.float8e4, tag="w2")
                                    eng = dmae[nd[0] % 2]
                                    nd[0] += 1
                                    eng.dma_start(out=t[:],
                                                  in_=w2s[f, :, i - 1])
                                    wts[i] = t
                            psm = {}
                            for i in ORDER:
                                ps = ps2.tile([128, 256], dt.float32,
                                              tag=f"m{i}")
                                psm[i] = ps
                                for blk in range(KBS):
                                    nc.tensor.matmul(
                                        ps[:],
                                        wts[i][:, blk],
                                        moving(i, blk),
                                        start=(blk == 0),
                                        stop=(blk == KBS - 1),
                                        perf_mode=DR,
                                    )
                                # DVE may read at most one PSUM operand per
                                # op, so M4/M5/M2 are staged to SBUF on the
                                # (otherwise idle) ACT engine first.
                                def ctile(tag):
                                    return cp2.tile([128, 256], dt.float32,
                                                    tag=tag, name=tag)
                                if i == 4:
                                    m4s = ctile("m4s")
                                    nc.scalar.activation(m4s[:], psm[4][:],
                                                         AF.Identity)
                                elif i == 7:
                                    x1 = ctile("x1")
                                    tt(x1[:], m4s[:], psm[7][:], ADD)
                                elif i == 5:
                                    m5s = ctile("m5s")
                                    nc.scalar.activation(m5s[:], psm[5][:],
                                                         AF.Identity)
                                elif i == 1:
                                    x2 = ctile("x2")
                                    tt(x2[:], x1[:], psm[1][:], ADD)
                                    c11 = ctile("c11")
                                    tt(c11[:], x2[:], m5s[:], SUB)
                                    nc.scalar.sign(h2[:, f, c0:c0 + 256],
                                                   c11[:],
                                                   bias=b2_sb[:, f:f + 1])
                                elif i == 2:
                                    c21 = ctile("c21")
                                    tt(c21[:], m4s[:], psm[2][:], ADD)
                                    nc.scalar.sign(
                                        h2[:, f, 512 + c0:512 + c0 + 256],
                                        c21[:], bias=b2_sb[:, f:f + 1])
                                    m2s = ctile("m2s")
                                    nc.scalar.activation(m2s[:], psm[2][:],
                                                         AF.Identity)
                                elif i == 3:
                                    c12 = ctile("c12")
                                    tt(c12[:], m5s[:], psm[3][:], ADD)
                                    nc.scalar.sign(
                                        h2[:, 24 + f, c0:c0 + 256],
                                        c12[:],
                                        bias=b2_sb[:, 24 + f:25 + f])
                                    y2 = ctile("y2")
                                    nc.vector.scalar_tensor_tensor(
                                        y2[:], m2s[:], -1.0, psm[1][:],
                                        MULT, ADD)
                                    y3 = ctile("y3")
                                    tt(y3[:], y2[:], psm[3][:], ADD)
                                elif i == 6:
                                    c22 = ctile("c22")
                                    tt(c22[:], y3[:], psm[6][:], ADD)
                                    nc.scalar.sign(
                                        h2[:, 24 + f,
                                           512 + c0:512 + c0 + 256],
                                        c22[:],
                                        bias=b2_sb[:, 24 + f:25 + f])

            # ---------------- fc3 + fused fc4 ----------------
            with tc.tile_pool(name="lgp", bufs=1, space="PSUM") as lgp, \
                 tc.tile_pool(name="lgsbp", bufs=1) as lgsbp:
                lg_psum = lgp.tile([DOUT, BC], dt.float32)
                with tc.tile_pool(name="w3pool", bufs=3) as w3pool, \
                     tc.tile_pool(name="h3pool", bufs=18) as h3pool, \
                     tc.tile_pool(name="ps3", bufs=3, space="PSUM") as ps3:
                    h3_tiles = [None] * MT

                    def fc4_mms(m):
                        t_h3 = h3_tiles[m]
                        for n in range(NH):
                            nc.tensor.matmul(
                                lg_psum[:, n * 512:(n + 1) * 512],
                                w4_sb[:, m, :],
                                t_h3[:, n * 512:(n + 1) * 512],
                                start=(m == 0),
                                stop=(m == MT - 1),
                            )

                    for m in range(MT):
                        if m == 0:
                            wsb = w3f
                        else:
                            wsb = w3pool.tile([128, KB, 128, 2],
                                              dt.float8e4, tag="w3")
                            nc.sync.dma_start(out=wsb[:], in_=w3p[m])
                        psum = ps3.tile([128, BC], dt.float32, tag="ps3")
                        for b in range(KB):
                            for n in range(NH):
                                nc.tensor.matmul(
                                    psum[:, n * 512:(n + 1) * 512],
                                    wsb[:, b],
                                    h2[:, 2 * b:2 * b + 2,
                                       n * 512:(n + 1) * 512],
                                    start=(b == 0),
                                    stop=(b == KB - 1),
                                    perf_mode=DRS,
                                )
                        t_h3 = h3pool.tile([128, BC], dt.float16, tag="h3")
                        nc.scalar.activation(t_h3[:], psum[:, :],
                                             AF.Identity,
                                             bias=b3_sb[:, m:m + 1])
                        nc.vector.tensor_scalar(t_h3[:], t_h3[:], 1.0, -1.0,
                                                mybir.AluOpType.min,
                                                mybir.AluOpType.max)
                        h3_tiles[m] = t_h3
                        # fc4 batched every 8 m-tiles, pipelined one m behind
                        if m % 8 == 7 and m >= 15:
                            for mm in range(m - 15, m - 7):
                                fc4_mms(mm)
                    for mm in range(MT - 8, MT):
                        fc4_mms(mm)

                # ------------- bias + log_softmax (max-free) -------------
                # logits are bounded (|l| < 40), so exp() cannot overflow
                # fp32 and the rowmax subtraction is unnecessary:
                # out = l - ln(sum(exp(l))). Partition 32 of lg_sb holds the
                # per-column ln-sum so one PE transpose carries both.
                lg_sb = lgsbp.tile([33, BC], dt.float32)
                nc.scalar.activation(lg_sb[0:DOUT, :], lg_psum[:],
                                     AF.Identity, bias=b4_sb[:, 0:1])
                NJ = BC // 128
                with tc.tile_pool(name="tp", bufs=1, space="PSUM") as tpp, \
                     tc.tile_pool(name="sm", bufs=1) as smp:
                    ex_sb = smp.tile([DOUT, BC], dt.float16, tag="ex")
                    nc.scalar.activation(ex_sb[:], lg_psum[:], AF.Exp,
                                         bias=b4_sb[:, 0:1])
                    sums_ps = tpp.tile([1, BC], dt.float32, tag="sums")
                    for n in range(NH):
                        nc.tensor.matmul(
                            sums_ps[:, n * 512:(n + 1) * 512],
                            ones_sb[:, 0:1],
                            ex_sb[:, n * 512:(n + 1) * 512],
                        )
                    nc.scalar.activation(lg_sb[32:33, :], sums_ps[:], AF.Ln)
                    for j in range(NJ):
                        tp = tpp.tile([128, 33], dt.float32, tag=f"tp{j%4}")
                        nc.tensor.transpose(
                            tp[:], lg_sb[:, j * 128:(j + 1) * 128], ident[:])
                        res = smp.tile([128, DOUT], dt.float32,
                                       tag=f"res{j}")
                        nc.vector.tensor_scalar(res[:], tp[:, 0:DOUT],
                                                tp[:, 32:33], None,
                                                mybir.AluOpType.subtract)
                        nc.sync.dma_start(
                            out=out[j * 128:(j + 1) * 128, :], in_=res[:])

    nc.compile()
    return nc


def _pack_inputs(x, w1, b1, w2, b2, w3, b3, w4, b4):
    """Host-side packing into the device layouts. Shared tensors are packed
    once; only xt differs per core."""
    f32 = np.float32
    f16 = np.float16
    x = np.asarray(x, f32).reshape(B, DIN)

    # fc1 weights: sign(w1).T stacked twice (hi/lo terms share the weights),
    # padded to [1664, 6144], layout [q, p, k, m]
    s1 = np.sign(np.asarray(w1, f32))                       # [DH, DIN]
    s1t = np.zeros((K1P, DH), f16)
    s1t[:DIN] = s1.T
    s1t[DIN:2 * DIN] = s1.T
    w1t = np.ascontiguousarray(
        s1t.reshape(KT1, 128, MQ, MPQ * 128).transpose(2, 1, 0, 3))

    # fc2 weights: Strassen T-combos of sign(w2).T, DoubleRow layout per
    # 128-wide output chunk: [fo, p, 7, blk, i2, f']
    s2m = np.sign(np.asarray(w2, f32)).T                    # [in, out] = B
    H = DH // 2
    B11 = s2m[:H, :H]
    B12 = s2m[:H, H:]
    B21 = s2m[H:, :H]
    B22 = s2m[H:, H:]
    Ts = [B11 + B22, B11, B12 - B22, B21 - B11, B22, B11 + B12, B21 + B22]

    def pack_t(t):   # [3072, 3072] -> [fo, p, blk, i2, f']
        r = t.reshape(KBS, 2, 128, NF, 128)
        return r.transpose(3, 2, 0, 1, 4)

    w2sp = np.ascontiguousarray(
        np.stack([pack_t(t) for t in Ts], axis=2)).astype(FP8)

    def pack_drsw(w):
        # DoubleRowSwInterleave stationary: per column j the (A,B) pair is
        # interleaved and columns run in reverse order: [mo, p, b, m', i]
        # with element [p, 2j+i] = W_i[p, 127-j]
        st = np.sign(np.asarray(w, f32)).T                  # [in, out]
        r = st.reshape(KB, 2, 128, MT, 128)                 # [b, i, p, mo, m']
        rev = r[..., ::-1]                                  # reverse m'
        return np.ascontiguousarray(rev.transpose(3, 2, 0, 4, 1)).astype(FP8)

    w3pk = pack_drsw(w3)

    # fc4 weights: w4.T in fp16, layout [p, j, c]
    w4t = np.asarray(w4, f32).T.astype(f16)                 # [DH, DOUT]
    w4pk = np.ascontiguousarray(w4t.reshape(MT, 128, DOUT).transpose(1, 0, 2))

    def pack_b(b):
        return np.ascontiguousarray(np.asarray(b, f32).reshape(MT, 128).T)

    b1pk, b2pk, b3pk = pack_b(b1), pack_b(b2), pack_b(b3)
    b4pk = np.asarray(b4, f32).reshape(DOUT, 1)

    shared = {"w1t": w1t, "w2s": w2sp, "w3p": w3pk, "w4p": w4pk,
              "b1p": b1pk, "b2p": b2pk, "b3p": b3pk, "b4p": b4pk}

    # per-core x: fp16 hi/lo split stacked along contraction, layout [p, k, n]
    in_maps = []
    for c in range(CORES):
        xc = x[c * BC:(c + 1) * BC]                         # [BC, DIN]
        hi = xc.astype(f16)
        lo = (xc - hi.astype(f32)).astype(f16)
        arr = np.zeros((K1P, BC), f16)
        arr[:DIN] = hi.T
        arr[DIN:2 * DIN] = lo.T
        xtc = np.ascontiguousarray(arr.reshape(KT1, 128, BC).transpose(1, 0, 2))
        in_maps.append({"xt": xtc, **shared})
    return in_maps


_cached_nc = None


def kernel(x, w1, b1, w2, b2, w3, b3, w4, b4):
    global _cached_nc, last_exec_time_ns
    import os
    trace = bool(int(os.environ.get("KERNEL_TRACE", "0")))
    if _cached_nc is None:
        _cached_nc = _build_program()
    in_maps = _pack_inputs(x, w1, b1, w2, b2, w3, b3, w4, b4)
    res = run_bass_kernel_spmd(_cached_nc, in_maps, list(range(CORES)),
                               trace=trace)
    last_exec_time_ns = res.exec_time_ns
    return np.concatenate([res.results[c]["out"] for c in range(CORES)], axis=0)
